# revision 1
# baseline (speedup 1.0000x reference)
"""GAT layer kernel for Trainium2 (8 NeuronCores, data-parallel over batch).

Reference computation (per graph b):
    Wh  = atoms @ W                      (N, FO)
    s1  = Wh @ a1 ; s2 = Wh @ a2         (N,)
    e   = leaky_relu(s1[:,None]+s2[None,:], 0.1)
    att = softmax(where(adj>0, e, -9e15), axis=1)
    out = elu(att @ Wh)

On-device formulation (no transcendental ever touches the NxN matrix):
    exp(leaky_relu(s)) = max(e^{s1_i} e^{s2_j}, e^{0.1 s1_i} e^{0.1 s2_j})
and because softmax row-normalizes, any per-row factor cancels, so with
r_i = min(e^{-0.9 s1_i}, 15000) (the clamp is row-uniform, hence exact):
    B_ij = max(v_j, r_i * q_j),  v = e^{s2-5}, q = e^{0.1 s2 - 5}
    att_ij = adj_ij B_ij / sum_j adj_ij B_ij
The 0/1 adjacency multiplies post-"exp" (exact: masked entries contribute 0
to numerator and denominator, equivalent to the reference's -9e15 trick).
The denominator comes free as a ones-column appended to Wh in the
P^T @ [Wh|1] matmul.

Layouts: score tiles are [i partitions, j free] (adjacency loads at line
rate); P blocks are PE-transposed (128x128) so the attention matmul
contracts j on partitions.  Score pipeline in fp16, everything else fp32.
"""

import numpy as np
from contextlib import ExitStack

import concourse.bass as bass
import concourse.tile as tile
import concourse.mybir as mybir
from concourse.masks import make_identity

dt = mybir.dt
Alu = mybir.AluOpType
Act = mybir.ActivationFunctionType

N = 1024          # nodes per graph
F_IN = 128        # input features
FO = 64           # output features
P = 128           # partitions
NCH = N // P      # 8 node chunks
N_CORES = 8
B_FULL = 64
M_SHIFT = 10.0    # exponent recentering; halves go into v and q


def build_gat(bpc: int, reps: int = 1) -> bass.Bass:
    """Emit the bass program for one core processing `bpc` graphs."""
    nc = bass.Bass()
    atoms = nc.declare_dram_parameter("atoms", [bpc, N, F_IN], dt.float32, isOutput=False)
    adj = nc.declare_dram_parameter("adj", [bpc, N, N], dt.int32, isOutput=False)
    wext = nc.declare_dram_parameter("wext", [F_IN, FO + 2], dt.float32, isOutput=False)
    selmat = nc.declare_dram_parameter("selmat", [NCH, NCH * P], dt.float32, isOutput=False)
    out = nc.declare_dram_parameter("out", [bpc, N, FO], dt.float32, isOutput=True)

    with tile.TileContext(nc) as tc, ExitStack() as ctx:
        consts = ctx.enter_context(tc.tile_pool(name="consts", bufs=1))
        psum = ctx.enter_context(tc.tile_pool(name="psum", bufs=6, space="PSUM"))
        psum2 = ctx.enter_context(tc.tile_pool(name="psum2", bufs=1, space="PSUM"))
        gbuf = ctx.enter_context(tc.tile_pool(name="gbuf", bufs=3))
        cbuf = ctx.enter_context(tc.tile_pool(name="cbuf", bufs=3))
        adjbuf = ctx.enter_context(tc.tile_pool(name="adjbuf", bufs=3))

        ident_f = consts.tile([P, P], dt.float32, tag="idf")
        make_identity(nc, ident_f)
        ident_b = consts.tile([P, P], dt.float16, tag="idb")
        make_identity(nc, ident_b)
        ones_b = consts.tile([1, P], dt.float16, tag="onb")
        nc.vector.memset(ones_b, 1.0)
        wext_sb = consts.tile([P, FO + 2], dt.float32, tag="wext")
        nc.sync.dma_start(out=wext_sb, in_=wext[:, :])
        bias_mh = consts.tile([P, 1], dt.float32, tag="bmh")
        nc.vector.memset(bias_mh, -M_SHIFT / 2)
        bias_z = consts.tile([P, 1], dt.float32, tag="bz")
        nc.vector.memset(bias_z, 0.0)
        # sel[:, c*P:(c+1)*P] is all-ones in row c: K=8 matmul with it as
        # stationary broadcasts row c of an [8, 128] tile to all partitions.
        sel_sb = consts.tile([NCH, NCH * P], dt.float16, tag="sel")
        nc.gpsimd.dma_start(out=sel_sb, in_=selmat[:, :])

        rep_tag = [0]

        def precompute(gg):
            g = f"{gg}_r{rep_tag[0]}"
            # ---------------- per-graph precompute (small) ----------------
            atoms_sb = gbuf.tile([P, NCH, F_IN], dt.float32, tag="atoms", name=f"atoms_{g}")
            nc.sync.dma_start(out=atoms_sb, in_=atoms[gg].rearrange("(c p) f -> p c f", p=P))

            # transpose atoms chunks: atT[:, c, :] = [feat, node]
            atT_sb = gbuf.tile([P, NCH, P], dt.float32, tag="atT", name=f"atT_{g}")
            atT_ps = psum2.tile([P, NCH, P], dt.float32, tag="ps2", name=f"atT_ps_{g}")
            for c in range(NCH):
                nc.tensor.transpose(atT_ps[:, c, :], atoms_sb[:, c, :], ident_f)
            nc.scalar.copy(out=atT_sb, in_=atT_ps)

            # [Wh | s1 | s2] = atoms_chunk @ [W | Wa1 | Wa2]
            whones = gbuf.tile([P, NCH, FO + 1], dt.float16, tag="whones", name=f"whones_{g}")
            nc.vector.memset(whones[:, :, FO:FO + 1], 1.0)
            s12 = gbuf.tile([P, NCH, 2], dt.float32, tag="s12", name=f"s12_{g}")
            for h in range(2):
                whc_ps = psum.tile([P, 4, FO + 2], dt.float32, tag="ps", name=f"whc_ps_{g}_{h}")
                for cc in range(4):
                    c = h * 4 + cc
                    nc.tensor.matmul(whc_ps[:, cc, :], lhsT=atT_sb[:, c, :], rhs=wext_sb,
                                     start=True, stop=True)
                nc.scalar.copy(out=whones[:, h * 4:(h + 1) * 4, 0:FO],
                               in_=whc_ps[:, :, 0:FO])
                nc.vector.tensor_copy(out=s12[:, h * 4:(h + 1) * 4, :],
                                      in_=whc_ps[:, :, FO:FO + 2])

            # r_i = exp(-0.9 s1) (f32 cols); v = exp(s2-20), q = exp(.1 s2-20)
            rraw = gbuf.tile([P, NCH], dt.float32, tag="rraw", name=f"rraw_{g}")
            nc.scalar.activation(rraw, s12[:, :, 0], Act.Exp, bias=bias_z, scale=-0.9)
            rcols = gbuf.tile([P, NCH], dt.float32, tag="rcols", name=f"rcols_{g}")
            nc.vector.tensor_scalar(rcols, rraw, 15000.0, None, Alu.min)
            vqcols = gbuf.tile([P, NCH, 2], dt.float16, tag="vqcols", name=f"vqcols_{g}")
            nc.scalar.activation(vqcols[:, :, 0], s12[:, :, 1], Act.Exp, bias=bias_mh, scale=1.0)
            nc.scalar.activation(vqcols[:, :, 1], s12[:, :, 1], Act.Exp, bias=bias_mh, scale=0.1)

            # transpose v,q cols -> rows, then broadcast across partitions
            vt_ps = psum.tile([NCH, P], dt.float16, tag="ps", name=f"vt_ps_{g}")
            nc.tensor.transpose(vt_ps, vqcols[:, :, 0], ident_b)
            qt_ps = psum.tile([NCH, P], dt.float16, tag="ps", name=f"qt_ps_{g}")
            nc.tensor.transpose(qt_ps, vqcols[:, :, 1], ident_b)
            vt_sb = gbuf.tile([NCH, P], dt.float16, tag="vqt", name=f"vt_{g}")
            nc.vector.tensor_copy(out=vt_sb, in_=vt_ps)
            qt_sb = gbuf.tile([NCH, P], dt.float16, tag="vqt2", name=f"qt_{g}")
            nc.vector.tensor_copy(out=qt_sb, in_=qt_ps)

            vb_sb = gbuf.tile([P, N], dt.float16, tag="vb", name=f"vb_{g}")
            qb_sb = gbuf.tile([P, N], dt.float16, tag="qb", name=f"qb_{g}")
            vb_ps = psum2.tile([P, N], dt.float32, tag="ps2", name=f"vb_ps_{g}")
            qb_ps = psum2.tile([P, N], dt.float32, tag="ps2", name=f"qb_ps_{g}")
            for c in range(NCH):
                nc.tensor.matmul(vb_ps[:, c * P:(c + 1) * P], lhsT=sel_sb[:, c * P:(c + 1) * P],
                                 rhs=vt_sb, start=True, stop=True)
                nc.tensor.matmul(qb_ps[:, c * P:(c + 1) * P], lhsT=sel_sb[:, c * P:(c + 1) * P],
                                 rhs=qt_sb, start=True, stop=True)
            nc.scalar.copy(out=vb_sb, in_=vb_ps)
            nc.scalar.copy(out=qb_sb, in_=qb_ps)

            res_g = gbuf.tile([P, NCH, FO], dt.float32, tag="res", name=f"res_{g}")
            return dict(whones=whones, rcols=rcols, vb=vb_sb, qb=qb_sb, res=res_g)

        def mainloop_pair(gg, hp, st):
            g = f"{gg}_r{rep_tag[0]}"
            whones, rcols, vb_sb, qb_sb, res_g = (
                st["whones"], st["rcols"], st["vb"], st["qb"], st["res"])
            ic0 = 2 * hp
            adj_sb = adjbuf.tile([P, 2, N], dt.int32, tag="adj", name=f"adj_{g}_{hp}")
            nc.sync.dma_start(
                out=adj_sb,
                in_=adj[gg, ic0 * P:(ic0 + 2) * P, :].rearrange("(c p) j -> p c j", p=P))
            adj_bf = cbuf.tile([P, 2, N], dt.float16, tag="adjbf", name=f"adjbf_{g}_{hp}")
            nc.gpsimd.tensor_copy(out=adj_bf, in_=adj_sb)

            # B = max(v_j, r_i q_j); P = B * adj
            em = cbuf.tile([P, 2, N], dt.float16, tag="em", name=f"em_{g}_{hp}")
            for k in range(2):
                ic = ic0 + k
                t2 = cbuf.tile([P, N], dt.float16, tag="t2", name=f"t2_{g}_{ic}")
                nc.vector.tensor_scalar(t2, qb_sb, rcols[:, ic:ic + 1], None, Alu.mult)
                nc.vector.tensor_tensor(em[:, k, :], vb_sb, t2, Alu.max)
            pm = cbuf.tile([P, 2, N], dt.float16, tag="pm", name=f"pm_{g}_{hp}")
            nc.vector.tensor_tensor(pm, em, adj_bf, Alu.mult)

            # transpose P blocks: pt[:, k, jc, :] = P_k[:, jc]^T  ([j, i])
            pt_sb = cbuf.tile([P, 2, NCH, P], dt.float16, tag="pt", name=f"pt_{g}_{hp}")
            for k in range(2):
                pt_ps = psum.tile([P, NCH, P], dt.float16, tag="ps", name=f"pt_ps_{g}_{hp}_{k}")
                for jc in range(NCH):
                    nc.tensor.transpose(pt_ps[:, jc, :], pm[:, k, jc * P:(jc + 1) * P], ident_b)
                nc.scalar.copy(out=pt_sb[:, k], in_=pt_ps)

            # h'[i, 0:64] + denom col: sum_j P^T[j,i] * [Wh|1][j,:]
            h_list = []
            for k in range(2):
                h_ps = psum.tile([P, FO + 1], dt.float32, tag="ps", name=f"h_ps_{g}_{hp}_{k}")
                for jc in range(NCH):
                    nc.tensor.matmul(h_ps, lhsT=pt_sb[:, k, jc, :],
                                     rhs=whones[:, jc, :],
                                     start=(jc == 0), stop=(jc == NCH - 1))
                h_list.append(h_ps)

            # finalize: divide by denom, ELU
            rec = cbuf.tile([P, 2], dt.float32, tag="rec", name=f"rec_{g}_{hp}")
            hdiv = cbuf.tile([P, 2, FO], dt.float32, tag="hdiv", name=f"hdiv_{g}_{hp}")
            for k in range(2):
                nc.vector.reciprocal(rec[:, k:k + 1], h_list[k][:, FO:FO + 1])
                nc.vector.tensor_scalar(hdiv[:, k, :], h_list[k][:, 0:FO],
                                        rec[:, k:k + 1], None, Alu.mult)
            hexp = cbuf.tile([P, 2, FO], dt.float32, tag="hexp", name=f"hexp_{g}_{hp}")
            nc.scalar.activation(hexp, hdiv, Act.Exp, bias=bias_z)
            em1 = cbuf.tile([P, 2, FO], dt.float32, tag="em1", name=f"em1_{g}_{hp}")
            nc.gpsimd.tensor_scalar(em1, hexp, -1.0, 0.0, Alu.add, Alu.min)
            nc.vector.tensor_tensor(res_g[:, ic0:ic0 + 2, :], hdiv, em1, Alu.max)

        def flush(gg, st):
            nc.sync.dma_start(out=out[gg].rearrange("(c p) f -> p c f", p=P), in_=st["res"])

        for _rep in range(reps):
          rep_tag[0] = _rep
          if bpc % 2 == 0:
            for gp in range(bpc // 2):
                g0, g1 = 2 * gp, 2 * gp + 1
                st0 = precompute(g0)
                st1 = precompute(g1)
                for hp in range(NCH // 2):
                    mainloop_pair(g0, hp, st0)
                    mainloop_pair(g1, hp, st1)
                flush(g0, st0)
                flush(g1, st1)
          else:
            for g in range(bpc):
                st = precompute(g)
                for hp in range(NCH // 2):
                    mainloop_pair(g, hp, st)
                flush(g, st)

    # HW allows at most one sync-wait per Matmult/Ldweights; Tile can emit
    # more.  Run the bacc lowering passes that move extra waits onto
    # ldweights / standalone event-semaphore instructions.
    import bass_rust as _br
    _br.move_matmul_waits_to_ldweights(nc.m)
    _br.generate_event_semaphores(nc)
    return nc


_NC_CACHE: dict[int, bass.Bass] = {}


def _get_nc(bpc: int) -> bass.Bass:
    if bpc not in _NC_CACHE:
        _NC_CACHE[bpc] = build_gat(bpc)
    return _NC_CACHE[bpc]


def _make_wext(W: np.ndarray, a: np.ndarray) -> np.ndarray:
    a1 = a[:FO, :]
    a2 = a[FO:, :]
    return np.concatenate([W, W @ a1, W @ a2], axis=1).astype(np.float32)


def _make_sel() -> np.ndarray:
    sel = np.zeros((NCH, NCH * P), dtype=np.float32)
    for c in range(NCH):
        sel[c, c * P:(c + 1) * P] = 1.0
    return sel


def kernel(atoms_vector: np.ndarray, adjacency: np.ndarray, W: np.ndarray,
           a: np.ndarray) -> np.ndarray:
    from concourse.bass_utils import run_bass_kernel_spmd

    B = atoms_vector.shape[0]
    bpc = B // N_CORES
    wext = _make_wext(W, a)
    sel = _make_sel()

    nc = _get_nc(bpc)
    in_maps = []
    for i in range(N_CORES):
        sl = slice(i * bpc, (i + 1) * bpc)
        in_maps.append({
            "atoms": np.ascontiguousarray(atoms_vector[sl]).astype(np.float32, copy=False),
            "adj": np.ascontiguousarray(adjacency[sl]).astype(np.int32, copy=False),
            "wext": wext,
            "selmat": sel,
        })
    res = run_bass_kernel_spmd(nc, in_maps, list(range(N_CORES)))
    return np.concatenate([res.results[i]["out"] for i in range(N_CORES)], axis=0)

